# revision 31
# baseline (speedup 1.0000x reference)
"""SAGAN-style attention block (nn_AttentionBlock) on 8 Trainium2 NeuronCores.

Problem (per sample): x [C=64, N=4096] (N = 64x64 spatial),
  f = Wf x + bf   [8, N]       g = Wg x + bg   [8, N]
  h = Wh x + bh   [64, N]
  s = f^T g       [N, N];  beta = softmax(s, axis=1)   (over j)
  o[c, i] = sum_j h[c, j] beta[i, j];   out = x + o

Sharding: pure data parallel over batch B=8 -> one sample per core, no
collectives. Full inputs in, full outputs out; host code only marshals.

Per-core algorithm (channels on SBUF partitions):
  - One consolidated input block [65, 4176]: x_aug = [x; ones] plus the
    transposed, bias-augmented weights (ones row + bias rows fold the
    biases into the matmuls). DMA'd in 5 pieces for overlap.
  - ~6us of full-array dummy PE matmuls run under the input DMA so the
    HAM clock gate opens (1.2 -> 2.4 GHz) before the real work (helps
    when HAM is the limiter; a chip-level thermal throttle sometimes
    holds the clock down regardless).
  - Projections on the PE: f/g in fp32 (their scores get exponentiated,
    so keep max precision there), hT in bf16 with x as the stationary
    operand, giving hT[j, c] tiles directly -- no transposes anywhere.
    f/g are replicated to partition offsets 0/32/64 (bf16 DVE copies)
    and hT gets a 65th all-ones column (computes the softmax
    denominator for free inside the o-matmul).
  - Main loop over 8 i-chunks (512 queries) x 32 j-tiles (128 keys),
    j-tiles in groups of 3:
      s-trio: three K=8 bf16 matmuls row-packed into PE quadrants
        0/32/64 (concurrent; ~390ns for all three) -> 3-bank PSUM macro
      e = exp(s - 12) on ACT in one [128, 1536] instruction, bf16 out
        (the -12 shift cancels in the division; pure overflow margin)
      o_aug[65, 512] += hT_j^T @ e_j, alternating between TWO PSUM
        accumulators (even/odd j) so consecutive accumulating matmuls
        pipeline instead of serializing on one bank.
    Emission is software-pipelined: the s-trio of group m+1 is emitted
    BEFORE the o-matmuls of group m, so the in-order PE queue runs it
    while ACT computes exp(m). Steady state is ACT-bound at the exp
    floor (~1.44us per group).
  - Finalize per i-chunk, entirely off the PE: DVE merges the two
    accumulator halves, reciprocal_approx_fast of the denominator row
    (copied to partition 0 first -- the custom DVE op misbehaves on
    nonzero base partitions), broadcast of 1/den across partitions via
    a DRAM round-trip DMA (stride-0 reads are only legal from DRAM),
    then divide-and-residual on DVE and a per-chunk output DMA.

PSUM budget: 3 + 3 (double-buffered s-macros) + 1 + 1 (o accumulators).
Numerics: bf16 scores give rel-L2 ~2.5e-3, absmax ~0.7% of output
scale; exp arguments up to ~+30 with fp32 range to spare.
"""
import sys

sys.path.insert(0, "/opt/trn_rl_repo")

import numpy as np
from contextlib import ExitStack

try:  # tracing hook is optional; provide a no-op stub if absent
    import antenv.axon_hooks  # noqa: F401
except ImportError:
    import types
    import antenv
    _stub = types.ModuleType("antenv.axon_hooks")
    _stub.get_axon_ntff_profile_hook = lambda: None
    _stub.set_axon_ntff_profile_hook = lambda hook: None
    sys.modules["antenv.axon_hooks"] = _stub
    antenv.axon_hooks = _stub

import concourse.bass as bass  # noqa: F401  (bacc subclasses Bass)
import concourse.tile as tile
from concourse import bacc, mybir
from concourse.bass_utils import run_bass_kernel_spmd

F32 = mybir.dt.float32
F32R = mybir.dt.float32r
BF16 = mybir.dt.bfloat16
MMDT = BF16   # dtype for the big s/o matmul operands

B, C, H, W = 8, 64, 64, 64
N = H * W          # 4096
C8 = 8             # f/g channels
NCORES = 8
ICHUNK = 512       # i (query) tile width; one PSUM bank
NI = N // ICHUNK   # 8
JT = 128           # j (key) tile = PSUM partitions
NJ = N // JT       # 32
GROUP = 3          # j-tiles per PSUM macro tile / per exp instruction

_CACHE = {}


def _build_nc():
    nc = bacc.Bacc("TRN2", target_bir_lowering=False, debug=False,
                   num_devices=NCORES)
    # input layout: [65, 4096 + 16 + 64]
    #   [:, 0:4096]      x_aug (row 64 = ones)
    #   [:, 4096:4112]   wfg_aug = [[Wf^T, Wg^T]; [bf, bg]]
    #   [:, 4112:4176]   wh_aug  = [Wh^T; bh]
    inp = nc.dram_tensor("inp", [C + 1, N + 16 + C], F32,
                         kind="ExternalInput").ap()
    out = nc.dram_tensor("out", [C, N], F32, kind="ExternalOutput").ap()

    with tile.TileContext(nc) as tc:
        with ExitStack() as ctx:
            sb = ctx.enter_context(tc.tile_pool(name="sb", bufs=1))
            epool = ctx.enter_context(tc.tile_pool(name="ep", bufs=8))
            fin = ctx.enter_context(tc.tile_pool(name="fin", bufs=8))
            psA = ctx.enter_context(tc.tile_pool(name="psA", bufs=1, space="PSUM"))
            psB = ctx.enter_context(tc.tile_pool(name="psB", bufs=1, space="PSUM"))
            psO1 = ctx.enter_context(tc.tile_pool(name="psO1", bufs=1, space="PSUM"))
            psO2 = ctx.enter_context(tc.tile_pool(name="psO2", bufs=1, space="PSUM"))
            dram = ctx.enter_context(tc.tile_pool(name="dram", bufs=8, space="DRAM"))

            # ---- constants (early, low DVE ticks) ----
            ones_f = sb.tile([128, 1], F32)
            nc.vector.memset(ones_f[:], 1.0)
            ones_b = sb.tile([1, 256], MMDT)
            nc.vector.tensor_copy(ones_b[:], ones_f[0:1, 0:1].to_broadcast((1, 256)))
            expbias = sb.tile([128, 1], F32)
            nc.vector.memset(expbias[:], -12.0)

            # ---- input: weights first, then x in quarters (overlap) ----
            tin = sb.tile([C + 1, N + 16 + C], F32)
            nc.sync.dma_start(tin[:, N:N + 16 + C], inp[:, N:N + 16 + C])
            for c in range(4):
                nc.sync.dma_start(tin[:, c * 1024:(c + 1) * 1024],
                                  inp[:, c * 1024:(c + 1) * 1024])
            x_aug = tin[:, 0:N]
            wfg = tin[:, N:N + 16]
            wh = tin[:, N + 16:N + 16 + C]

            pools4 = [(psA, "m"), (psB, "m"), (psO1, "po1"), (psO2, "po2")]

            # ---- PE warmup: ~6us of full-array dummy matmuls while the
            # input DMA runs, so the HAM activity monitor sees a sustained
            # busy window and opens the clock gate (1.2 -> 2.4 GHz).
            # Reads an uninitialized SBUF tile; the result is never read. ----
            warm_src = sb.tile([JT, ICHUNK], MMDT)
            nc.vector.tensor_copy(warm_src[:],
                                  ones_f[:, 0:1].to_broadcast((JT, ICHUNK)))
            warm_ps = psO1.tile([JT, 256], F32, tag="po1", name="warmps")
            for _ in range(12):
                nc.tensor.matmul(warm_ps[:], warm_src[:, 0:JT],
                                 warm_src[:, 0:256], start=True, stop=True)

            # bf16 copies of x_aug and the weights: all projections run
            # with bf16 operands (single-pass matmuls, cheap LDWEIGHTS).
            # The extra f/g input-quantization noise adds ~sqrt(2) to the
            # existing bf16 score noise -- measured rel-L2 stays ~3e-3.
            xb = sb.tile([C + 1, N], MMDT)
            for c in range(4):
                if c % 2 == 0:
                    nc.vector.tensor_copy(xb[:, c * 1024:(c + 1) * 1024],
                                          x_aug[:, c * 1024:(c + 1) * 1024])
                else:
                    nc.scalar.copy(xb[:, c * 1024:(c + 1) * 1024],
                                   x_aug[:, c * 1024:(c + 1) * 1024])
            whb = sb.tile([C + 1, C], MMDT)
            nc.vector.tensor_copy(whb[:], wh)
            # f32r (12-bit mantissa) copies for the f/g projections: same
            # matmul speed as bf16 at N=512, 16x less input-quantization
            # noise on the scores that get exponentiated
            xr = sb.tile([C + 1, N], F32R)
            for c in range(4):
                if c % 2 == 1:
                    nc.vector.tensor_copy(xr[:, c * 1024:(c + 1) * 1024],
                                          x_aug[:, c * 1024:(c + 1) * 1024])
                else:
                    nc.scalar.copy(xr[:, c * 1024:(c + 1) * 1024],
                                   x_aug[:, c * 1024:(c + 1) * 1024])
            wfgr = sb.tile([C + 1, 16], F32R)
            nc.vector.tensor_copy(wfgr[:], wfg)

            # ---- projections: f and g first (the main loop needs them
            # immediately); replicate to partition offsets 32/64 via fast
            # bf16 DVE copies for the 3x row-packed s-matmuls ----
            f_sb = sb.tile([72, N], MMDT)
            g_sb = sb.tile([72, N], MMDT)
            for c in range(NI):
                cs = slice(c * ICHUNK, (c + 1) * ICHUNK)
                xc = xr[:, cs]
                _pl, _tg = pools4[c % 4]
                ppf = _pl.tile([C8, ICHUNK], F32, tag=_tg, name=f"ppf{c}")
                nc.tensor.matmul(ppf[:], wfgr[:, 0:C8], xc, start=True, stop=True)
                nc.scalar.copy(f_sb[0:C8, cs], ppf[:])
                _pl, _tg = pools4[(c + 2) % 4]
                ppg = _pl.tile([C8, ICHUNK], F32, tag=_tg, name=f"ppg{c}")
                nc.tensor.matmul(ppg[:], wfgr[:, C8:16], xc, start=True, stop=True)
                nc.vector.tensor_copy(g_sb[0:C8, cs], ppg[:])
                # per-chunk replication (bf16 SBUF->SBUF, overlaps the
                # projection pipeline instead of serializing at the end)
                for r in (32, 64):
                    nc.vector.tensor_copy(f_sb[r:r + C8, cs], f_sb[0:C8, cs])
                    nc.vector.tensor_copy(g_sb[r:r + C8, cs], g_sb[0:C8, cs])

            # ---- projections: hT (4 j-tiles per PSUM bank, one evac each) ----
            hT = sb.tile([JT, NJ, C + 1], MMDT)
            for t4 in range(NJ // 4):
                _pl, _tg = pools4[t4 % 4]
                pp = _pl.tile([JT, 4 * C], F32, tag=_tg, name=f"pph{t4}")
                for u in range(4):
                    t = 4 * t4 + u
                    nc.tensor.matmul(pp[:, u * C:(u + 1) * C],
                                     xb[:, t * JT:(t + 1) * JT], whb[:],
                                     start=True, stop=True)
                nc.vector.tensor_copy(
                    hT[:, 4 * t4:4 * t4 + 4, 0:C],
                    pp[:].rearrange("p (a b) -> p a b", a=4))
            nc.vector.tensor_copy(hT[:, :, C:C + 1],
                                  ones_f[:].to_broadcast((JT, NJ, 1)))

            # ---- main attention loop ----
            res = sb.tile([C, N], F32)    # final output staging
            groups = []
            j0 = 0
            while j0 < NJ:
                groups.append((j0, min(GROUP, NJ - j0)))
                j0 += GROUP

            def emit_o(po, e, j0, glen):
                for k in range(glen):
                    j = j0 + k
                    nc.tensor.matmul(
                        po[j % 2][:], hT[:, j, :],
                        e[:, k * ICHUNK:(k + 1) * ICHUNK],
                        start=(j < 2), stop=(j >= NJ - 2))

            def fin_front(po, q):
                # merge the two o-accumulator halves ASAP (frees the PSUM
                # banks for the next chunk's o-matmuls), then launch the
                # reciprocal + DRAM-bounce broadcast of 1/den
                oc = fin.tile([C + 1, ICHUNK], F32, tag="oc",
                              name=f"oc{q}")
                nc.vector.tensor_copy(oc[:], po[0][:])
                nc.vector.tensor_add(oc[:], oc[:], po[1][:])
                dn = fin.tile([1, ICHUNK], F32, tag="dn", name=f"dn{q}")
                nc.vector.tensor_copy(dn[:], oc[C:C + 1, :])
                r = fin.tile([1, ICHUNK], F32, tag="r", name=f"r{q}")
                nc.vector.reciprocal_approx_fast(r[:], dn[:])
                scr = dram.tile([1, ICHUNK], F32, tag="scr", name=f"scr{q}")
                nc.sync.dma_start(scr[:], r[:])
                rb = fin.tile([C, ICHUNK], F32, tag="rb", name=f"rb{q}")
                nc.sync.dma_start(rb[:], scr[:].to_broadcast((C, ICHUNK)))
                return (oc, rb, q)

            def fin_back(oc, rb, q):
                # divide + residual + output DMA. Emitted one chunk late so
                # the DVE never queues behind the rb DMA round trip (FIFO
                # engine: a waiting mult would block the next chunk's merge
                # and stall its o-matmuls on the accumulator WAR).
                qs = slice(q * ICHUNK, (q + 1) * ICHUNK)
                nc.vector.tensor_mul(res[:, qs], oc[0:C, :], rb[:])
                nc.vector.tensor_add(res[:, qs], res[:, qs], tin[0:C, qs])
                nc.sync.dma_start(out[:, qs], res[:, qs])

            # Software-pipelined emission: the o-matmuls of group m are
            # emitted AFTER the s-trio of group m+1, so the in-order PE
            # queue can run that s-trio while ACT is still computing
            # exp(m) (o-matmuls of m must wait for it).
            gidx = 0          # global group counter for A/B alternation
            pend_o = None     # (po, e, j0, glen) of the previous group
            pend_fin = None   # (po, q) once a chunk's last o is emitted
            pend_back = None  # deferred divide/residual of the prior chunk
            po = None
            for q in range(NI):
                qs = slice(q * ICHUNK, (q + 1) * ICHUNK)
                po1 = psO1.tile([C + 1, ICHUNK], F32, tag="po1", name=f"po1_{q}")
                po2 = psO2.tile([C + 1, ICHUNK], F32, tag="po2", name=f"po2_{q}")
                po = [po1, po2]
                for gi, (j0, glen) in enumerate(groups):
                    pool = psA if gidx % 2 == 0 else psB
                    gidx += 1
                    pm = pool.tile([JT, GROUP * ICHUNK], F32, tag="m")
                    for k in range(glen):
                        j = j0 + k
                        nc.tensor.matmul(
                            pm[:, k * ICHUNK:(k + 1) * ICHUNK],
                            g_sb[32 * k:32 * k + C8, j * JT:(j + 1) * JT],
                            f_sb[32 * k:32 * k + C8, qs],
                            start=True, stop=True)
                    e = epool.tile([JT, GROUP * ICHUNK], MMDT, tag="e")
                    nc.scalar.activation(e[:, 0:glen * ICHUNK],
                                         pm[:, 0:glen * ICHUNK],
                                         mybir.ActivationFunctionType.Exp,
                                         bias=expbias[:])
                    if pend_o is not None:
                        emit_o(*pend_o)
                    if pend_fin is not None:
                        front = fin_front(*pend_fin)
                        pend_fin = None
                        if pend_back is not None:
                            fin_back(*pend_back)
                        pend_back = front
                    pend_o = (po, e, j0, glen)
                pend_fin = (po, q)
            emit_o(*pend_o)
            front = fin_front(*pend_fin)
            if pend_back is not None:
                fin_back(*pend_back)
            fin_back(*front)
    nc.compile()
    return nc


def _marshal(x_b, Wf, bf, Wg, bg, Wh, bh):
    """Build the per-core [65, 4176] input block."""
    xa = np.empty((C + 1, N + 16 + C), dtype=np.float32)
    xa[0:C, 0:N] = x_b.reshape(C, N)
    xa[C, 0:N] = 1.0
    xa[0:C, N:N + C8] = Wf.T
    xa[C, N:N + C8] = bf
    xa[0:C, N + C8:N + 16] = Wg.T
    xa[C, N + C8:N + 16] = bg
    xa[0:C, N + 16:N + 16 + C] = Wh.T
    xa[C, N + 16:N + 16 + C] = bh
    return xa


LAST_RESULTS = None


def kernel(x, Wf, bf, Wg, bg, Wh, bh):
    global LAST_RESULTS
    x = np.asarray(x, dtype=np.float32)
    Wf = np.asarray(Wf, dtype=np.float32)
    bf = np.asarray(bf, dtype=np.float32)
    Wg = np.asarray(Wg, dtype=np.float32)
    bg = np.asarray(bg, dtype=np.float32)
    Wh = np.asarray(Wh, dtype=np.float32)
    bh = np.asarray(bh, dtype=np.float32)

    if "nc" not in _CACHE:
        _CACHE["nc"] = _build_nc()
    nc = _CACHE["nc"]

    in_maps = [{"inp": _marshal(x[b], Wf, bf, Wg, bg, Wh, bh)}
               for b in range(NCORES)]
    res = run_bass_kernel_spmd(nc, in_maps, list(range(NCORES)))
    LAST_RESULTS = res
    out = np.stack([res.results[b]["out"] for b in range(NCORES)], axis=0)
    return out.reshape(B, C, H, W).astype(np.float32)


# revision 32
# speedup vs baseline: 1.1945x; 1.1945x over previous
"""SAGAN-style attention block (nn_AttentionBlock) on 8 Trainium2 NeuronCores.

Problem (per sample): x [C=64, N=4096] (N = 64x64 spatial),
  f = Wf x + bf   [8, N]       g = Wg x + bg   [8, N]
  h = Wh x + bh   [64, N]
  s = f^T g       [N, N];  beta = softmax(s, axis=1)   (over j)
  o[c, i] = sum_j h[c, j] beta[i, j];   out = x + o

Sharding: pure data parallel over batch B=8 -> one sample per core, no
collectives. Full inputs in, full outputs out; host code only marshals.

Per-core algorithm (channels on SBUF partitions):
  - One consolidated input block [65, 4176]: x_aug = [x; ones] plus the
    transposed, bias-augmented weights (ones row + bias rows fold the
    biases into the matmuls). DMA'd in 5 pieces for overlap.
  - ~6us of full-array dummy PE matmuls run under the input DMA so the
    HAM clock gate opens (1.2 -> 2.4 GHz) before the real work (helps
    when HAM is the limiter; a chip-level thermal throttle sometimes
    holds the clock down regardless).
  - Projections on the PE: f/g in float32r (12-bit mantissa at bf16
    matmul speed -- their scores get exponentiated, so precision there
    matters most), hT in bf16 with x as the stationary operand, giving
    hT[j, c] tiles directly -- no transposes anywhere.
    f/g are replicated to partition offsets 0/32/64 (bf16 DVE copies)
    and hT gets a 65th all-ones column (computes the softmax
    denominator for free inside the o-matmul).
  - Main loop over 8 i-chunks (512 queries) x 32 j-tiles (128 keys),
    j-tiles in groups of 3:
      s-trio: three K=8 bf16 matmuls row-packed into PE quadrants
        0/32/64 (concurrent; ~390ns for all three) -> 3-bank PSUM macro
      e = exp(s - 12) on ACT in one [128, 1536] instruction, bf16 out
        (the -12 shift cancels in the division; pure overflow margin)
      o_aug[65, 512] += hT_j^T @ e_j, alternating between TWO PSUM
        accumulators (even/odd j) so consecutive accumulating matmuls
        pipeline instead of serializing on one bank.
    Emission is software-pipelined: the s-trio of group m+1 is emitted
    BEFORE the o-matmuls of group m, so the in-order PE queue runs it
    while ACT computes exp(m). Steady state is ACT-bound at the exp
    floor (~1.44us per group).
  - Finalize per i-chunk, entirely off the PE: DVE merges the two
    accumulator halves, reciprocal_approx_fast of the denominator row
    (copied to partition 0 first -- the custom DVE op misbehaves on
    nonzero base partitions), broadcast of 1/den across partitions via
    a DRAM round-trip DMA (stride-0 reads are only legal from DRAM),
    then divide-and-residual on DVE and a per-chunk output DMA.

PSUM budget: 3 + 3 (double-buffered s-macros) + 1 + 1 (o accumulators).
Numerics: bf16 scores give rel-L2 ~2.5e-3, absmax ~1.2% of output
scale; exp arguments up to ~+30 with fp32 range to spare.
"""
import sys

sys.path.insert(0, "/opt/trn_rl_repo")

import numpy as np
from contextlib import ExitStack

try:  # tracing hook is optional; provide a no-op stub if absent
    import antenv.axon_hooks  # noqa: F401
except ImportError:
    import types
    import antenv
    _stub = types.ModuleType("antenv.axon_hooks")
    _stub.get_axon_ntff_profile_hook = lambda: None
    _stub.set_axon_ntff_profile_hook = lambda hook: None
    sys.modules["antenv.axon_hooks"] = _stub
    antenv.axon_hooks = _stub

import concourse.bass as bass  # noqa: F401  (bacc subclasses Bass)
import concourse.tile as tile
from concourse import bacc, mybir
from concourse.bass_utils import run_bass_kernel_spmd

F32 = mybir.dt.float32
F32R = mybir.dt.float32r
BF16 = mybir.dt.bfloat16
MMDT = BF16   # dtype for the big s/o matmul operands

B, C, H, W = 8, 64, 64, 64
N = H * W          # 4096
C8 = 8             # f/g channels
NCORES = 8
ICHUNK = 512       # i (query) tile width; one PSUM bank
NI = N // ICHUNK   # 8
JT = 128           # j (key) tile = PSUM partitions
NJ = N // JT       # 32
GROUP = 3          # j-tiles per PSUM macro tile / per exp instruction

_CACHE = {}


def _build_nc():
    nc = bacc.Bacc("TRN2", target_bir_lowering=False, debug=False,
                   num_devices=NCORES)
    # input layout: [65, 4096 + 16 + 64]
    #   [:, 0:4096]      x_aug (row 64 = ones)
    #   [:, 4096:4112]   wfg_aug = [[Wf^T, Wg^T]; [bf, bg]]
    #   [:, 4112:4176]   wh_aug  = [Wh^T; bh]
    inp = nc.dram_tensor("inp", [C + 1, N + 16 + C], F32,
                         kind="ExternalInput").ap()
    out = nc.dram_tensor("out", [C, N], F32, kind="ExternalOutput").ap()

    with tile.TileContext(nc) as tc:
        with ExitStack() as ctx:
            sb = ctx.enter_context(tc.tile_pool(name="sb", bufs=1))
            epool = ctx.enter_context(tc.tile_pool(name="ep", bufs=8))
            fin = ctx.enter_context(tc.tile_pool(name="fin", bufs=8))
            psA = ctx.enter_context(tc.tile_pool(name="psA", bufs=1, space="PSUM"))
            psB = ctx.enter_context(tc.tile_pool(name="psB", bufs=1, space="PSUM"))
            psO1 = ctx.enter_context(tc.tile_pool(name="psO1", bufs=1, space="PSUM"))
            psO2 = ctx.enter_context(tc.tile_pool(name="psO2", bufs=1, space="PSUM"))
            dram = ctx.enter_context(tc.tile_pool(name="dram", bufs=8, space="DRAM"))

            # ---- constants (early, low DVE ticks) ----
            ones_f = sb.tile([128, 1], F32)
            nc.vector.memset(ones_f[:], 1.0)
            ones_b = sb.tile([1, 256], MMDT)
            nc.vector.tensor_copy(ones_b[:], ones_f[0:1, 0:1].to_broadcast((1, 256)))
            expbias = sb.tile([128, 1], F32)
            nc.vector.memset(expbias[:], -12.0)

            # ---- input: weights first, then x in quarters (overlap) ----
            tin = sb.tile([C + 1, N + 16 + C], F32)
            nc.sync.dma_start(tin[:, N:N + 16 + C], inp[:, N:N + 16 + C])
            for c in range(4):
                nc.sync.dma_start(tin[:, c * 1024:(c + 1) * 1024],
                                  inp[:, c * 1024:(c + 1) * 1024])
            x_aug = tin[:, 0:N]
            wfg = tin[:, N:N + 16]
            wh = tin[:, N + 16:N + 16 + C]

            pools4 = [(psA, "m"), (psB, "m"), (psO1, "po1"), (psO2, "po2")]

            # ---- PE warmup: ~6us of full-array dummy matmuls while the
            # input DMA runs, so the HAM activity monitor sees a sustained
            # busy window and opens the clock gate (1.2 -> 2.4 GHz).
            # Reads an uninitialized SBUF tile; the result is never read. ----
            warm_src = sb.tile([JT, ICHUNK], MMDT)
            nc.vector.tensor_copy(warm_src[:],
                                  ones_f[:, 0:1].to_broadcast((JT, ICHUNK)))
            warm_ps = psO1.tile([JT, 256], F32, tag="po1", name="warmps")
            for _ in range(12):
                nc.tensor.matmul(warm_ps[:], warm_src[:, 0:JT],
                                 warm_src[:, 0:256], start=True, stop=True)

            # bf16 copies of x_aug and the weights: all projections run
            # with bf16 operands (single-pass matmuls, cheap LDWEIGHTS).
            # The extra f/g input-quantization noise adds ~sqrt(2) to the
            # existing bf16 score noise -- measured rel-L2 stays ~3e-3.
            xb = sb.tile([C + 1, N], MMDT)
            for c in range(4):
                if c % 2 == 0:
                    nc.vector.tensor_copy(xb[:, c * 1024:(c + 1) * 1024],
                                          x_aug[:, c * 1024:(c + 1) * 1024])
                else:
                    nc.scalar.copy(xb[:, c * 1024:(c + 1) * 1024],
                                   x_aug[:, c * 1024:(c + 1) * 1024])
            whb = sb.tile([C + 1, C], MMDT)
            nc.vector.tensor_copy(whb[:], wh)
            # f32r (12-bit mantissa) copies for the f/g projections: same
            # matmul speed as bf16 at N=512, 16x less input-quantization
            # noise on the scores that get exponentiated
            xr = sb.tile([C + 1, N], F32R)
            for c in range(4):
                if c % 2 == 1:
                    nc.vector.tensor_copy(xr[:, c * 1024:(c + 1) * 1024],
                                          x_aug[:, c * 1024:(c + 1) * 1024])
                else:
                    nc.scalar.copy(xr[:, c * 1024:(c + 1) * 1024],
                                   x_aug[:, c * 1024:(c + 1) * 1024])
            wfgr = sb.tile([C + 1, 16], F32R)
            nc.vector.tensor_copy(wfgr[:], wfg)

            # ---- projections: f and g first (the main loop needs them
            # immediately); replicate to partition offsets 32/64 via fast
            # bf16 DVE copies for the 3x row-packed s-matmuls ----
            f_sb = sb.tile([72, N], MMDT)
            g_sb = sb.tile([72, N], MMDT)
            for c in range(NI):
                cs = slice(c * ICHUNK, (c + 1) * ICHUNK)
                xc = xr[:, cs]
                _pl, _tg = pools4[c % 4]
                ppf = _pl.tile([C8, ICHUNK], F32, tag=_tg, name=f"ppf{c}")
                nc.tensor.matmul(ppf[:], wfgr[:, 0:C8], xc, start=True, stop=True)
                nc.scalar.copy(f_sb[0:C8, cs], ppf[:])
                _pl, _tg = pools4[(c + 2) % 4]
                ppg = _pl.tile([C8, ICHUNK], F32, tag=_tg, name=f"ppg{c}")
                nc.tensor.matmul(ppg[:], wfgr[:, C8:16], xc, start=True, stop=True)
                nc.vector.tensor_copy(g_sb[0:C8, cs], ppg[:])
                # per-chunk replication (bf16 SBUF->SBUF, overlaps the
                # projection pipeline instead of serializing at the end)
                for r in (32, 64):
                    nc.vector.tensor_copy(f_sb[r:r + C8, cs], f_sb[0:C8, cs])
                    nc.vector.tensor_copy(g_sb[r:r + C8, cs], g_sb[0:C8, cs])

            # ---- projections: hT (4 j-tiles per PSUM bank, one evac each) ----
            hT = sb.tile([JT, NJ, C + 1], MMDT)
            for t4 in range(NJ // 4):
                _pl, _tg = pools4[t4 % 4]
                pp = _pl.tile([JT, 4 * C], F32, tag=_tg, name=f"pph{t4}")
                for u in range(4):
                    t = 4 * t4 + u
                    nc.tensor.matmul(pp[:, u * C:(u + 1) * C],
                                     xb[:, t * JT:(t + 1) * JT], whb[:],
                                     start=True, stop=True)
                nc.vector.tensor_copy(
                    hT[:, 4 * t4:4 * t4 + 4, 0:C],
                    pp[:].rearrange("p (a b) -> p a b", a=4))
            nc.vector.tensor_copy(hT[:, :, C:C + 1],
                                  ones_f[:].to_broadcast((JT, NJ, 1)))

            # ---- main attention loop ----
            res = sb.tile([C, N], F32)    # final output staging
            groups = []
            j0 = 0
            while j0 < NJ:
                groups.append((j0, min(GROUP, NJ - j0)))
                j0 += GROUP

            def emit_o(po, e, j0, glen):
                for k in range(glen):
                    j = j0 + k
                    nc.tensor.matmul(
                        po[j % 2][:], hT[:, j, :],
                        e[:, k * ICHUNK:(k + 1) * ICHUNK],
                        start=(j < 2), stop=(j >= NJ - 2))

            def fin_front(po, q):
                # merge the two o-accumulator halves ASAP (frees the PSUM
                # banks for the next chunk's o-matmuls), then launch the
                # reciprocal + DRAM-bounce broadcast of 1/den
                oc = fin.tile([C + 1, ICHUNK], F32, tag="oc",
                              name=f"oc{q}")
                nc.vector.tensor_copy(oc[:], po[0][:])
                nc.vector.tensor_add(oc[:], oc[:], po[1][:])
                dn = fin.tile([1, ICHUNK], F32, tag="dn", name=f"dn{q}")
                nc.vector.tensor_copy(dn[:], oc[C:C + 1, :])
                r = fin.tile([1, ICHUNK], F32, tag="r", name=f"r{q}")
                nc.vector.reciprocal_approx_fast(r[:], dn[:])
                scr = dram.tile([1, ICHUNK], F32, tag="scr", name=f"scr{q}")
                nc.sync.dma_start(scr[:], r[:])
                rb = fin.tile([C, ICHUNK], F32, tag="rb", name=f"rb{q}")
                nc.sync.dma_start(rb[:], scr[:].to_broadcast((C, ICHUNK)))
                return (oc, rb, q)

            def fin_back(oc, rb, q):
                # divide + residual + output DMA. Emitted one chunk late so
                # the DVE never queues behind the rb DMA round trip (FIFO
                # engine: a waiting mult would block the next chunk's merge
                # and stall its o-matmuls on the accumulator WAR).
                qs = slice(q * ICHUNK, (q + 1) * ICHUNK)
                nc.vector.tensor_mul(res[:, qs], oc[0:C, :], rb[:])
                nc.vector.tensor_add(res[:, qs], res[:, qs], tin[0:C, qs])
                nc.sync.dma_start(out[:, qs], res[:, qs])

            # Software-pipelined emission: the o-matmuls of group m are
            # emitted AFTER the s-trio of group m+1, so the in-order PE
            # queue can run that s-trio while ACT is still computing
            # exp(m) (o-matmuls of m must wait for it).
            gidx = 0          # global group counter for A/B alternation
            pend_o = None     # (po, e, j0, glen) of the previous group
            pend_fin = None   # (po, q) once a chunk's last o is emitted
            pend_back = None  # deferred divide/residual of the prior chunk
            po = None
            for q in range(NI):
                qs = slice(q * ICHUNK, (q + 1) * ICHUNK)
                po1 = psO1.tile([C + 1, ICHUNK], F32, tag="po1", name=f"po1_{q}")
                po2 = psO2.tile([C + 1, ICHUNK], F32, tag="po2", name=f"po2_{q}")
                po = [po1, po2]
                for gi, (j0, glen) in enumerate(groups):
                    pool = psA if gidx % 2 == 0 else psB
                    gidx += 1
                    pm = pool.tile([JT, GROUP * ICHUNK], F32, tag="m")
                    for k in range(glen):
                        j = j0 + k
                        nc.tensor.matmul(
                            pm[:, k * ICHUNK:(k + 1) * ICHUNK],
                            g_sb[32 * k:32 * k + C8, j * JT:(j + 1) * JT],
                            f_sb[32 * k:32 * k + C8, qs],
                            start=True, stop=True)
                    e = epool.tile([JT, GROUP * ICHUNK], MMDT, tag="e")
                    nc.scalar.activation(e[:, 0:glen * ICHUNK],
                                         pm[:, 0:glen * ICHUNK],
                                         mybir.ActivationFunctionType.Exp,
                                         bias=expbias[:])
                    if pend_o is not None:
                        emit_o(*pend_o)
                    if pend_fin is not None:
                        front = fin_front(*pend_fin)
                        pend_fin = None
                        if pend_back is not None:
                            fin_back(*pend_back)
                        pend_back = front
                    pend_o = (po, e, j0, glen)
                pend_fin = (po, q)
            emit_o(*pend_o)
            front = fin_front(*pend_fin)
            if pend_back is not None:
                fin_back(*pend_back)
            fin_back(*front)
    nc.compile()
    return nc


def _marshal(x_b, Wf, bf, Wg, bg, Wh, bh):
    """Build the per-core [65, 4176] input block."""
    xa = np.empty((C + 1, N + 16 + C), dtype=np.float32)
    xa[0:C, 0:N] = x_b.reshape(C, N)
    xa[C, 0:N] = 1.0
    xa[0:C, N:N + C8] = Wf.T
    xa[C, N:N + C8] = bf
    xa[0:C, N + C8:N + 16] = Wg.T
    xa[C, N + C8:N + 16] = bg
    xa[0:C, N + 16:N + 16 + C] = Wh.T
    xa[C, N + 16:N + 16 + C] = bh
    return xa


LAST_RESULTS = None


def kernel(x, Wf, bf, Wg, bg, Wh, bh):
    global LAST_RESULTS
    x = np.asarray(x, dtype=np.float32)
    Wf = np.asarray(Wf, dtype=np.float32)
    bf = np.asarray(bf, dtype=np.float32)
    Wg = np.asarray(Wg, dtype=np.float32)
    bg = np.asarray(bg, dtype=np.float32)
    Wh = np.asarray(Wh, dtype=np.float32)
    bh = np.asarray(bh, dtype=np.float32)

    if "nc" not in _CACHE:
        _CACHE["nc"] = _build_nc()
    nc = _CACHE["nc"]

    in_maps = [{"inp": _marshal(x[b], Wf, bf, Wg, bg, Wh, bh)}
               for b in range(NCORES)]
    res = run_bass_kernel_spmd(nc, in_maps, list(range(NCORES)))
    LAST_RESULTS = res
    out = np.stack([res.results[b]["out"] for b in range(NCORES)], axis=0)
    return out.reshape(B, C, H, W).astype(np.float32)
